# revision 84
# baseline (speedup 1.0000x reference)
"""Fused LayerNorm + MHA + out-proj for Trainium2, SPMD across 8 NeuronCores.

Problem: x[2,2048,1024] -> LN -> qkv (w_qkv[1024,3072]) -> 16-head attention
(dim_head 64) -> out proj (w_out[1024,1024] + b_out).

Sharding: core c handles batch c//4 and head-quad c%4 (heads 4*(c%4)..+4).
Each core: LN + transpose of its batch (replicated within the batch group),
qkv for its 4 heads, full attention for its 4 (b,h) pairs, then AllToAlls
redistribute head outputs under STRIPED row ownership: core c owns rows
[128c, 128c+128) u [1024+128c, 1024+128c+128) of EACH batch (one stripe per
query-chunk half), so every core's chunk addressing is identical (SPMD-safe)
AND pair-1's exchange splits into two half-size collectives - the A half
(rows 0:1024) fires after query-chunk 1 and hides under qc2/3 attention;
only the 21.5us B half is exposed. Each core then computes the final
projection for its 512 rows locally.

Key techniques: bf16 end-to-end data flow (host pre-casts x/w_qkv/w_out to
bf16: halves input DMA, full-rate transposes, 2x/4x DVE modes); feature-major
(transposed) layout; j-major qkv schedule with an INCREMENTAL attention
start (pair-0/qc-0 score tile-pairs interleave with the remaining LN/qkv
groups chunk by chunk) and the remaining qkv accumulation groups emitted
between attention query-chunks so independent matmuls fill the PE stalls
left by the exp dependency chain; softmax exp on ScalarE straight out of
PSUM with the 1/sqrt(d) scale folded in; no max-subtraction (scores are
O(5) sigma); 3/8 of exps on DVE via a one-op Schraudolph bf16 bit-trick;
single-bank score psum tiles in a 4-deep ring so a slot frees per exp;
softmax denominator via a fused ones-column in the attn@v stationary operand
(psum row 0 = sum of exp), reciprocal broadcast across partitions with a PE
ones-matmul (GpSimd must stay out of the attention dependency chain: any
post-collective instruction with a Pool-semaphore wait is conservatively
gated on the collective itself); pair-0's AllToAll overlaps pair-1
attention and pair-1's A-half collective overlaps its own qc2/3; out-proj
pass-1 + pass-2's A-half groups + PE-warm-up dummy matmuls run during the
exposed B-half collective so the final pass-2 groups start at full PE clock
(the p-state model halves throughput for ~3us after any idle); b_out bias
matmuls elided when the host sees b_out == 0; bf16 output DMA with
host-side upcast and output DMAs spread across three engine queues.
"""
import sys
sys.path.insert(0, '/opt/trn_rl_repo')
import numpy as np
import ml_dtypes

import concourse.bass as bass
import concourse.tile as tile
import concourse.mybir as mybir
from concourse import bacc
from concourse.bass_utils import run_bass_kernel_spmd
from concourse.masks import make_identity

F32 = mybir.dt.float32
BF16 = mybir.dt.bfloat16
AF = mybir.ActivationFunctionType
ALU = mybir.AluOpType

N_CORES = 8
B, N, DIM = 2, 2048, 1024
HEADS, DHEAD = 16, 64
H_LOC = 4                    # heads per core
ROWS = N                     # rows per core (one batch)
DT = DIM // 128              # 8 dim tiles
RCHUNK = 512
N_CH = ROWS // RCHUNK        # 4 row chunks
NKT = N // 128               # 16 key tiles
NQC = N // 512               # 4 query chunks
SCALE = DHEAD ** -0.5
EPS = 1e-5
# bf16-space Schraudolph fast exp: bitcast_bf16(int16(s*A + B)) ~ exp(SCALE*s)
A_SCH = SCALE * 128.0 / float(np.log(2.0))
B_SCH = 127.0 * 128.0 - 7.5
I16 = mybir.dt.int16

_CACHED_NC = {}


def build(with_bias):
    nc = bacc.Bacc("TRN2", target_bir_lowering=False, debug=False,
                   num_devices=N_CORES)
    x_ext = nc.dram_tensor("x", [ROWS, DIM], BF16, kind="ExternalInput")
    gamma_ext = nc.dram_tensor("gamma", [DIM], F32, kind="ExternalInput")
    beta_ext = nc.dram_tensor("beta", [DIM], F32, kind="ExternalInput")
    wqkv_ext = nc.dram_tensor("wqkv", [DIM, 3 * H_LOC * DHEAD], BF16,
                              kind="ExternalInput")
    wout_ext = nc.dram_tensor("wout", [DIM, DIM], BF16, kind="ExternalInput")
    if with_bias:
        bout_ext = nc.dram_tensor("bout", [DIM], F32, kind="ExternalInput")
    out_ext = nc.dram_tensor("out", [RCHUNK, DIM], BF16, kind="ExternalOutput")

    with tile.TileContext(nc) as tc:
        with tc.tile_pool(name="singles", bufs=1) as singles, \
             tc.tile_pool(name="xin", bufs=3) as xin, \
             tc.tile_pool(name="xc", bufs=8) as xcp, \
             tc.tile_pool(name="lnxt", bufs=1) as lnxtp, \
             tc.tile_pool(name="stats", bufs=8) as statsp, \
             tc.tile_pool(name="exps", bufs=4) as expsp, \
             tc.tile_pool(name="div", bufs=2) as divp, \
             tc.tile_pool(name="osb", bufs=2) as osbp, \
             tc.tile_pool(name="dram", bufs=1, space="DRAM") as dram:

            # ---------------- constants / weights ----------------
            ident_f = singles.tile([128, 128], F32)
            make_identity(nc, ident_f)
            ident = singles.tile([128, 128], BF16)
            nc.vector.tensor_copy(ident, ident_f)

            # gamma/beta as [128, DT] (partition p, dim tile dt -> dim dt*128+p)
            gamma_sb = singles.tile([128, DT], F32)
            beta_sb = singles.tile([128, DT], F32)
            # scalar-engine DMA queue: keeps the SP queue's first dispatch
            # slot for chunk-0's x tiles (serial-bus order follows dispatch
            # completion order)
            nc.scalar.dma_start(out=gamma_sb,
                                in_=gamma_ext.ap().rearrange("(dt p) -> p dt", p=128))
            nc.scalar.dma_start(out=beta_sb,
                                in_=beta_ext.ap().rearrange("(dt p) -> p dt", p=128))

            eps_sb = singles.tile([128, 1], F32)
            nc.vector.memset(eps_sb, EPS)
            # preload ONLY the Sqrt ACT table while the engine idles waiting
            # for chunk-0's x DMA: the first rstd otherwise eats the 1.28us
            # table load on the LN critical path
            act_warm = singles.tile([1, 1], F32)
            nc.scalar.activation(out=act_warm, in_=eps_sb[0:1, :],
                                 func=AF.Sqrt, bias=0.0, scale=1.0)

            # w_qkv tile declared here; its DMA is issued after the first two
            # x chunks so those win the serial DMA bus (k-matmuls only need
            # w_sb at ~10us)
            w_sb = singles.tile([128, DT, 3 * H_LOC * DHEAD], BF16)

            ones_bf = singles.tile([1, 128], BF16)
            nc.vector.memset(ones_bf, 1.0)
            if with_bias:
                bo_sb = singles.tile([1, DIM], BF16)
                nc.gpsimd.dma_start(out=bo_sb, in_=bout_ext.ap().unsqueeze(0))

            # persistent activations
            qT = [singles.tile([128, ROWS], BF16, name=f"qT{i}") for i in range(2)]
            kT = [singles.tile([128, ROWS], BF16, name=f"kT{i}") for i in range(2)]
            # v_aug[:, h, t, 0]=1.0 (softmax denom), cols 1:64 zero pad,
            # [:, h, t, 64:128]=v -> attn@v psum row 0 = denom, rows 64:128 =
            # head outputs (64-aligned partition base for DVE access)
            # per-pair v_aug tiles: pair 0's zero/one padding comes from
            # early GpSimd memsets (all its attn@v consumers are emitted
            # before the first collective, so their conservative Pool-sem
            # wait is satisfied immediately); pair 1's padding is an ACT
            # copy of pair 0's zeroed region, leaving pair-1 attention with
            # NO GpSimd dependency (a Pool-sem wait from a post-collective
            # instruction would stall until the collective completes)
            v_augs = [singles.tile([128, 2, NKT, 128], BF16, name=f"v_aug{i}")
                      for i in range(2)]
            nc.gpsimd.memset(v_augs[0][:, :, :, 0:64], 0.0)
            nc.gpsimd.memset(v_augs[0][:, :, :, 0:1], 1.0)
            # head outputs (transposed): pair X holds heads 2X, 2X+1 in free dim
            oh = [singles.tile([128, 2, ROWS], BF16, name=f"oh{i}") for i in range(2)]

            # psum pools span LN/qkv/attention/pass-1: scores get a dedicated
            # ring (tag sc: 2 x 2 banks) so interleaved qkv/out-proj groups
            # (tag aux: 2 x 1 bank) never stall the exp pipeline; attn@v
            # accumulators tag ov: 2 x 1 bank. Total 8 banks.
            ps_sc_cm = tc.tile_pool(name="ps_sc", bufs=4, space="PSUM")
            ps_aux_cm = tc.tile_pool(name="ps_aux", bufs=2, space="PSUM")
            ps_ov_cm = tc.tile_pool(name="psov", bufs=2, space="PSUM")
            ps_sc = ps_sc_cm.__enter__()
            ps_aux = ps_aux_cm.__enter__()
            ps_ov = ps_ov_cm.__enter__()

            # ------------- phase 1a: LN + transpose, one chunk -------------
            lnxts = [None] * N_CH

            def ln_chunk(ch):
                xcs = []
                # one DMA per 512-row chunk: a single dispatch slot on the
                # SP sequencer, so chunk 0 beats the w_qkv transfer onto the
                # serial DMA bus. Chunk 0's first 128-row tile is split out
                # so its bn_stats start ~2us earlier.
                x_t4 = xin.tile([128, 4, DIM], BF16)
                nc.sync.dma_start(
                    out=x_t4,
                    in_=x_ext.ap()[ch * RCHUNK:(ch + 1) * RCHUNK, :]
                        .rearrange("(rt p) d -> p rt d", p=128))
                for rt in range(4):
                    x_t = x_t4[:, rt, :]
                    st = statsp.tile([128, 2, 6], F32, tag="bn")
                    for sg in range(2):
                        nc.vector.bn_stats(out=st[:, sg, :],
                                           in_=x_t[:, sg * 512:(sg + 1) * 512])
                    mv = statsp.tile([128, 2], F32, tag="mv")
                    nc.vector.bn_aggr(out=mv, in_=st)
                    rstd = statsp.tile([128, 1], F32, tag="rstd")
                    nc.scalar.activation(out=rstd, in_=mv[:, 1:2], func=AF.Sqrt,
                                         bias=eps_sb, scale=1.0)
                    nc.vector.reciprocal(out=rstd, in_=rstd)
                    xc = xcp.tile([128, DIM], BF16)
                    nc.vector.tensor_scalar(out=xc, in0=x_t,
                                            scalar1=mv[:, 0:1], scalar2=rstd,
                                            op0=ALU.subtract, op1=ALU.mult)
                    xcs.append(xc)
                lnxt = lnxtp.tile([128, DT, RCHUNK], BF16, name=f"lnxt{ch}")
                for db in range(DT):
                    tr_ps = ps_aux.tile([128, RCHUNK], BF16, tag="aux")
                    for rt in range(4):
                        nc.tensor.transpose(tr_ps[:, rt * 128:(rt + 1) * 128],
                                            xcs[rt][:, db * 128:(db + 1) * 128],
                                            ident)
                    # LN affine (gamma, beta are per-partition here); on ACT
                    # to keep DVE off the PE critical path
                    nc.scalar.activation(out=lnxt[:, db, :], in_=tr_ps,
                                         func=AF.Identity,
                                         bias=beta_sb[:, db:db + 1],
                                         scale=gamma_sb[:, db:db + 1])
                lnxts[ch] = lnxt

            # ------------- phase 1b: one qkv accumulation group -------------
            def qkv_group(ch, X, j):
                """j: 0=k, 1=q, 2=v for pair X, row chunk ch."""
                jc = [1, 0, 2][j] * 2 + X  # column order in w_sb: q0 q1 k0 k1 v0 v1
                sl = slice(ch * RCHUNK, (ch + 1) * RCHUNK)
                if j < 2:
                    qkv_ps = ps_aux.tile([128, RCHUNK], F32, tag="aux",
                                         name=f"qkv_ps_{ch}_{X}_{j}")
                    for db in range(DT):
                        nc.tensor.matmul(
                            qkv_ps,
                            w_sb[:, db, jc * 128:(jc + 1) * 128],
                            lnxts[ch][:, db, :],
                            start=(db == 0), stop=(db == DT - 1))
                    dst = (kT[X] if j == 0 else qT[X])[:, sl]
                    nc.scalar.copy(dst, qkv_ps)
                    return
                # v computed row-major directly (lhsT = lnxt slice, moving =
                # w_v): no transposes, and each row-tile gets a short
                # independent drain that clears the DVE queue early — the
                # attn@v stationary loads must not wait behind attention exps
                for rt in range(4):
                    t = ch * 4 + rt
                    v_ps = ps_aux.tile([128, 128], F32, tag="aux",
                                       name=f"v_ps_{ch}_{X}_{rt}")
                    for db in range(DT):
                        nc.tensor.matmul(
                            v_ps,
                            lnxts[ch][:, db, rt * 128:(rt + 1) * 128],
                            w_sb[:, db, jc * 128:(jc + 1) * 128],
                            start=(db == 0), stop=(db == DT - 1))
                    nc.vector.tensor_copy(
                        v_augs[X][:, 0:2, t, 64:128],
                        v_ps.rearrange("p (h d) -> p h d", h=2))

            # ------------- attention for one (pair, query chunk) -----------
            def attn_open(X, qc):
                return [ps_ov.tile([128, 512], F32, name=f"ov_{X}_{qc}_{i}",
                                   tag="ov") for i in range(2)]

            def attn_part(X, qc, o_ps, tp_lo, tp_hi):
                qsl = slice(qc * 512, (qc + 1) * 512)
                for tp in range(tp_lo, tp_hi):  # key-tile pairs
                    # one single-bank psum tile per (head, key-tile): the
                    # 4-deep ring frees a slot per exp instead of per pair,
                    # halving the exp-latency exposure on the score pipeline
                    s_ps = [[ps_sc.tile([128, 512], F32,
                                        name=f"sc_{X}_{qc}_{tp}_{i}_{ti}",
                                        tag="sc")
                             for ti in range(2)] for i in range(2)]
                    for ti in range(2):
                        t = tp * 2 + ti
                        ksl = slice(t * 128, (t + 1) * 128)
                        nc.tensor.matmul(s_ps[0][ti],
                                         kT[X][0:64, ksl], qT[X][0:64, qsl],
                                         start=True, stop=True,
                                         tile_position=(0, 0))
                        nc.tensor.matmul(s_ps[1][ti],
                                         kT[X][64:128, ksl], qT[X][64:128, qsl],
                                         start=True, stop=True,
                                         tile_position=(64, 0))
                    for i in range(2):  # head within pair
                        sch = (2 * tp + i) % 8 in (1, 3, 4, 6)
                        for ti in range(2):
                            t = tp * 2 + ti
                            if sch:
                                # one-op DVE fast exp (Schraudolph, bf16
                                # space): int16 output bits bitcast to bf16
                                # are exp(SCALE*s); offloads 3/8 of the exp
                                # work from the saturated ScalarE
                                ex_i = expsp.tile([128, 512], I16, tag="exi",
                                                  bufs=8)
                                nc.vector.tensor_scalar(
                                    out=ex_i, in0=s_ps[i][ti],
                                    scalar1=A_SCH, scalar2=B_SCH,
                                    op0=ALU.mult, op1=ALU.add)
                                ex = ex_i.bitcast(BF16)
                            else:
                                ex = expsp.tile([128, 512], BF16, tag="exb",
                                                bufs=8)
                                nc.scalar.activation(out=ex, in_=s_ps[i][ti],
                                                     func=AF.Exp,
                                                     bias=0.0, scale=SCALE)
                            nc.tensor.matmul(o_ps[i],
                                             v_augs[X][:, i, t, 0:128],
                                             ex,
                                             start=(t == 0), stop=(t == NKT - 1))

            def attn_finish(X, qc, o_ps):
                qsl = slice(qc * 512, (qc + 1) * 512)
                # divide by softmax denominator (row 0 of psum). The
                # partition broadcast is a PE ones-matmul, NOT GpSimd
                # partition_broadcast: pair-1 divides are emitted after the
                # first collective (a GpSimd-queue instruction) and their
                # conservative Pool-semaphore wait would stall them until
                # the collective completes
                for i in range(2):
                    r1 = divp.tile([1, 512], BF16, tag="r1")
                    with nc.allow_low_precision(
                            reason="softmax denom reciprocal in bf16: "
                                   "0.4% rms on a well-conditioned divisor"):
                        nc.vector.reciprocal(out=r1, in_=o_ps[i][0:1, :])
                    rb = ps_aux.tile([128, 512], F32, tag="aux",
                                     name=f"rb_{X}_{qc}_{i}")
                    nc.tensor.matmul(rb, ones_bf, r1, start=True, stop=True)
                    # drain raw head outputs to SBUF on ACT (overlaps the
                    # recip/broadcast) so the divide reads only one PSUM
                    # operand
                    oraw = divp.tile([64, 512], BF16, tag="oraw")
                    nc.scalar.copy(oraw, o_ps[i][64:128, :])
                    nc.vector.tensor_tensor(out=oh[X][64:128, i, qsl],
                                            in0=oraw,
                                            in1=rb[64:128, :], op=ALU.mult)
                # per-qc A2A input assembly under the STRIPED row ownership:
                # core c owns rows [128c, +128) u [1024+128c, +128) of each
                # batch, so qc 0/1 fill the A-half of every slot and qc 2/3
                # the B-half. Pair 1 stages A and B in separate buffers so
                # its collective can run as two half-size AllToAlls, the
                # first hidden under qc2/3 attention.
                half = qc // 2
                for jj in range(4):
                    j = 4 * (qc % 2) + jj
                    rows = slice(1024 * half + 128 * j,
                                 1024 * half + 128 * j + 128)
                    if X == 0:
                        dst = a2a_in0[j, :, :, half * 128:(half + 1) * 128]
                    else:
                        dst = a2a_in1[half][j, :, :, :]
                    nc.sync.dma_start(
                        out=dst.rearrange("i d r -> d i r"),
                        in_=oh[X][64:128, :, rows])

            def attn_qc(X, qc):
                o_ps = attn_open(X, qc)
                attn_part(X, qc, o_ps, 0, NKT // 2)
                attn_finish(X, qc, o_ps)

            # ------------- out-proj: one accumulation group -----------------
            e_parts = {}

            def op_group(gi, pair, ps_pool, psum_tag):
                """Rows rt = gi//2, out cols oc = gi%2, contraction over the
                pair's 4 inner tiles."""
                rt, oc = divmod(gi, 2)
                ep = ps_pool.tile([128, 512], F32, tag=psum_tag,
                                  name=f"op{pair}_{gi}")
                nmm = DT // 2
                for q in range(nmm):
                    nc.tensor.matmul(
                        ep,
                        outT[pair][:, q, rt // 2,
                                   (rt % 2) * 128:(rt % 2) * 128 + 128],
                        wo_sb[:, q * 2 + pair, oc * 512:(oc + 1) * 512],
                        start=(q == 0),
                        stop=(q == nmm - 1) and not (with_bias and pair == 0))
                if with_bias and pair == 0:
                    nc.tensor.matmul(
                        ep, ones_bf, bo_sb[:, oc * 512:(oc + 1) * 512],
                        start=False, stop=True)
                return ep

            def pass1_group(gi):
                ep = op_group(gi, 0, ps_aux, "aux")
                e_sb = osbp.tile([128, 512], F32, tag="e_sb", bufs=8,
                                 name=f"e_sb_{gi}")
                nc.scalar.copy(e_sb, ep)
                e_parts[gi] = e_sb

            # ---------------- emission schedule ----------------
            # pair 0: one full A2A (hidden under pair-1 attention). pair 1:
            # two half-size A2As (A = rows 0:1024 by stripe, B = 1024:2048);
            # A fires after qc1 and hides under qc2/3, so only the 21.5us
            # B half is exposed, and pass-2's A-half groups run during B.
            a2a_in0 = dram.tile([8, 2, 64, 256], BF16, name="a2a_in0")
            a2a_in1 = [dram.tile([8, 2, 64, 128], BF16, name=f"a2a_in1{h}")
                       for h in range(2)]
            a2a_out0 = dram.tile([8, 128, 256], BF16, name="a2a_out0")
            a2a_out1 = [dram.tile([8, 128, 128], BF16, name=f"a2a_out1{h}")
                        for h in range(2)]
            outT = [singles.tile([128, DT // 2, 2, 256], BF16, name=f"outT{i}")
                    for i in range(2)]

            ln_chunk(0)
            # w_qkv loads as three column-block DMAs (k, then v, then q)
            # interleaved with the x chunks on the serial DMA bus, so each
            # consumer group unblocks as early as possible; SWDGE descriptor
            # generation runs on an otherwise-empty Pool queue
            wq_re = wqkv_ext.ap().rearrange("(dt p) c -> p dt c", p=128)
            nc.gpsimd.dma_start(out=w_sb[:, :, 256:512], in_=wq_re[:, :, 256:512])
            ln_chunk(1)
            nc.gpsimd.dma_start(out=w_sb[:, :, 512:768], in_=wq_re[:, :, 512:768])
            nc.gpsimd.dma_start(out=w_sb[:, :, 0:256], in_=wq_re[:, :, 0:256])
            # incremental start of pair-0 qc=0 attention: scores for key
            # tiles of chunk ch only need k0/v0 of that chunk, so the first
            # attention tile-pairs interleave with the remaining LN/qkv work
            # instead of the PE idling until kT[0] completes
            qkv_group(0, 0, 0)            # k0 ch0
            qkv_group(0, 0, 2)            # v0 ch0
            qkv_group(0, 0, 1)            # q0 qc=0
            o00 = attn_open(0, 0)
            attn_part(0, 0, o00, 0, 2)    # key tiles 0..3
            qkv_group(1, 0, 0)
            qkv_group(1, 0, 2)
            ln_chunk(2)
            attn_part(0, 0, o00, 2, 4)
            qkv_group(2, 0, 0)
            qkv_group(2, 0, 2)
            ln_chunk(3)
            nc.scalar.copy(v_augs[1][:, :, :, 0:64], v_augs[0][:, :, :, 0:64])
            attn_part(0, 0, o00, 4, 6)
            qkv_group(3, 0, 0)            # kT[0] complete
            qkv_group(3, 0, 2)            # v0: v_aug[0:2] complete
            # w_out bf16 on the sync queue AFTER all x tiles: starts ~12us in,
            # done long before the out-proj needs it, off the startup critical
            # path
            wo_sb = singles.tile([128, DT, DIM], BF16)
            nc.sync.dma_start(
                out=wo_sb,
                in_=wout_ext.ap().rearrange("(it p) c -> p it c", p=128))
            attn_part(0, 0, o00, 6, 8)
            attn_finish(0, 0, o00)
            qkv_group(1, 0, 1)            # q0 qc=1

            # pair-1 k/v/q groups are spread through pair-0 attention as PE
            # stall filler, finishing well before pair-1 attention so their
            # psum->SBUF drains clear the ACT/DVE queues in time
            attn_qc(0, 1)
            qkv_group(2, 0, 1)
            qkv_group(0, 1, 0)            # k1 ch0
            qkv_group(0, 1, 2)            # v1 ch0
            attn_qc(0, 2)
            qkv_group(3, 0, 1)            # qT[0] complete
            qkv_group(1, 1, 0)
            qkv_group(1, 1, 2)
            qkv_group(2, 1, 0)
            qkv_group(3, 1, 0)            # kT[1] complete
            qkv_group(2, 1, 2)
            qkv_group(3, 1, 2)            # v1: v_aug[2:4] complete
            qkv_group(0, 1, 1)            # q1 qc=0
            attn_qc(0, 3)
            qkv_group(1, 1, 1)
            qkv_group(2, 1, 1)

            nc.gpsimd.collective_compute(
                "AllToAll", ALU.bypass,
                replica_groups=[[0, 1, 2, 3, 4, 5, 6, 7]],
                ins=[a2a_in0.opt()], outs=[a2a_out0.opt()])
            attn_qc(1, 0)
            qkv_group(3, 1, 1)            # qT[1] complete
            attn_qc(1, 1)
            # pair-1 A-half collective: its slots filled entirely by qc0/qc1,
            # so it runs hidden under qc2/qc3 attention
            nc.gpsimd.collective_compute(
                "AllToAll", ALU.bypass,
                replica_groups=[[0, 1, 2, 3, 4, 5, 6, 7]],
                ins=[a2a_in1[0].opt()], outs=[a2a_out1[0].opt()])
            for H in range(2):
                nc.gpsimd.dma_start(
                    out=outT[1][:, :, H, 0:128],
                    in_=a2a_out1[0][4 * H:4 * (H + 1)].rearrange(
                        "q p r -> p q r"))
            attn_qc(1, 2)
            attn_qc(1, 3)

            nc.gpsimd.collective_compute(
                "AllToAll", ALU.bypass,
                replica_groups=[[0, 1, 2, 3, 4, 5, 6, 7]],
                ins=[a2a_in1[1].opt()], outs=[a2a_out1[1].opt()])
            # B-half assembly on the scalar queue (waits only the B
            # collective; keeps the SWDGE queue clear)
            for H in range(2):
                nc.scalar.dma_start(
                    out=outT[1][:, :, H, 128:256],
                    in_=a2a_out1[1][4 * H:4 * (H + 1)].rearrange(
                        "q p r -> p q r"))
            # pair-0 outT assembly + all pass-1 out-proj groups + pass-2's
            # A-half groups run during the exposed B-half collective
            for H in range(2):
                nc.sync.dma_start(
                    out=outT[0][:, :, H, :],
                    in_=a2a_out0[4 * H:4 * (H + 1)].rearrange(
                        "q p r -> p q r"))
            for gi in range(8):
                pass1_group(gi)

            def pass2_group(gi):
                rt, oc = divmod(gi, 2)
                op_ps = op_group(gi, 1, ps_aux, "aux")
                o_sb = osbp.tile([128, 512], BF16, tag="o_sb", bufs=4)
                nc.vector.tensor_tensor(out=o_sb, in0=op_ps,
                                        in1=e_parts[gi], op=ALU.add)
                # spread output DMAs across engine queues so the per-queue
                # HWDGE dispatch overhead doesn't serialize the drain
                dq = [nc.sync, nc.scalar, nc.gpsimd][gi % 3]
                dq.dma_start(
                    out=out_ext.ap()[rt * 128:(rt + 1) * 128,
                                     oc * 512:(oc + 1) * 512], in_=o_sb)

            # A-half out-proj (stripe rows already exchanged)
            for gi in (0, 1, 4, 5):
                pass2_group(gi)
            # PE warm-up for the remainder of the B-half collective: the
            # p-state model halves matmul throughput for ~3us after any idle
            # period, so keep the clock up with throwaway matmuls sized to
            # end just before the B half lands
            warm_ps = ps_aux.tile([128, 512], F32, tag="aux", name="warm")
            for wi in range(83):
                nc.tensor.matmul(warm_ps, ident, wo_sb[:, 0, 0:512],
                                 start=True, stop=True)
            for gi in (2, 3, 6, 7):
                pass2_group(gi)
            ps_ov_cm.__exit__(None, None, None)
            ps_aux_cm.__exit__(None, None, None)
            ps_sc_cm.__exit__(None, None, None)

    nc.compile()
    return nc


def _make_in_maps(inputs, with_bias):
    bf16 = ml_dtypes.bfloat16
    x = np.ascontiguousarray(
        np.asarray(inputs["x"], dtype=np.float32).reshape(B * N, DIM)
    ).astype(bf16)
    gamma = np.asarray(inputs["gamma"], dtype=np.float32)
    beta = np.asarray(inputs["beta"], dtype=np.float32)
    w_qkv = np.asarray(inputs["w_qkv"], dtype=np.float32)
    w_out = np.ascontiguousarray(
        np.asarray(inputs["w_out"], dtype=np.float32)).astype(bf16)
    b_out = np.asarray(inputs["b_out"], dtype=np.float32)

    in_maps = []
    for c in range(N_CORES):
        b = c // 4
        qd = c % 4
        cols = []
        for j in range(3):
            cols.append(w_qkv[:, j * DIM + qd * 256:(j * DIM + qd * 256) + 256])
        wqkv_s = np.ascontiguousarray(np.concatenate(cols, axis=1)).astype(bf16)
        m = dict(
            x=np.ascontiguousarray(x[b * N:(b + 1) * N]),
            gamma=gamma, beta=beta,
            wqkv=wqkv_s, wout=w_out)
        if with_bias:
            m["bout"] = b_out
        in_maps.append(m)
    return in_maps


def kernel(x, gamma, beta, w_qkv, w_out, b_out):
    with_bias = bool(np.any(np.asarray(b_out) != 0))
    if with_bias not in _CACHED_NC:
        _CACHED_NC[with_bias] = build(with_bias)
    nc = _CACHED_NC[with_bias]
    in_maps = _make_in_maps(dict(x=x, gamma=gamma, beta=beta, w_qkv=w_qkv,
                                 w_out=w_out, b_out=b_out), with_bias)
    res = run_bass_kernel_spmd(nc, in_maps, core_ids=list(range(N_CORES)))
    # core c's "out" [512, 1024] under striped ownership: per batch H the
    # 256 rows are [A-stripe rows 128c..128c+128 | B-stripe 1024+128c..+128]
    out = np.empty((B, N, DIM), dtype=np.float32)
    for c in range(N_CORES):
        o = np.asarray(res.results[c]["out"]).astype(np.float32)
        for b in range(B):
            out[b, 128 * c:128 * (c + 1)] = o[b * 256:b * 256 + 128]
            out[b, 1024 + 128 * c:1024 + 128 * (c + 1)] = \
                o[b * 256 + 128:b * 256 + 256]
    return out
